# revision 3
# baseline (speedup 1.0000x reference)
"""DeepseekV3 top-k router kernel for 8x Trainium2 NeuronCores.

Strategy (v2):
  - Token dim (8192) sharded 8 ways; router weight replicated per core.
  - logits = hidden @ W.T decomposed as xh*wh + xh*wl + xl*w:
      * one 512-wide fp16 matmul per k-tile streams [wh | wl*2^11] through
        stationary xh -> psum[0:256] (hi) and psum[256:512] (cross1*2^11).
      * one fp8e4 DoubleRow matmul per k-PAIR contracts xl8 (=xl*2^16) against
        w8 (=w*2^11) for two k-tiles at once -> psum_dr (cross2*2^27).
    Bias b is folded in exactly via a 57th k-tile (ones row x [b16|b_res*2^11]).
    Empirically (fixed-seed inputs) this matches the fp32 reference's top-k
    indices as well as a full 3-pass fp16 scheme (1 near-tie row in 8192).
  - sigmoid + grouped top-k + weight extraction run on ACT/DVE per 128-token
    tile, overlapped with the next tile's matmuls. Weight gather uses
    scalar_tensor_tensor accumulate: eb[idx_j] = sum((sfc==m8_j)*eb).
  - DMA: x ships as fp16 hi (2B) + fp8 lo (1B); descriptors spread across
    sync/scalar/gpsimd engine queues so startup is not serialized on one
    sequencer.
"""

import numpy as np
import ml_dtypes

import concourse.bacc as bacc
import concourse.mybir as mybir
from concourse.tile import TileContext
from concourse import bass_utils

H = 7168
E = 256
T = 8192
NCORES = 8
TLOC = T // NCORES          # 1024 tokens per core
MT = 128                    # tokens per tile (PSUM partition dim)
NM = TLOC // MT             # 8 token tiles per core
KT = H // 128               # 56 contraction tiles
KP = KT + 1                 # +1 bias tile for the fp16 pass
K2 = KT // 2                # 28 DoubleRow k-pairs
TOP_K = 8
N_GROUP = 8
TOPK_GROUP = 4
EG = E // N_GROUP           # 32 experts per group
SCALE = 2.5
SCW = 2048.0                # 2^11 wl pre-scale (fp16 cross half)
SX8 = 2.0 ** 16             # xl -> fp8 pre-scale
SW8 = 2.0 ** 11             # w -> fp8 pre-scale
DSC = 1.0 / (SX8 * SW8)     # descale for the DoubleRow psum

f32 = mybir.dt.float32
f16 = mybir.dt.float16
f8 = mybir.dt.float8e4
u32 = mybir.dt.uint32
i32 = mybir.dt.int32
AOT = mybir.AluOpType
ACTF = mybir.ActivationFunctionType
DR = mybir.MatmulPerfMode.DoubleRow
NPF8 = ml_dtypes.float8_e4m3

_PROG = None

# per-m xh chunk sizes in k-tiles (sum = KP = 57)
XH_CH0 = [2, 6, 8, 8, 8, 8, 8, 9]     # m == 0 (fine-grained startup)
XH_CH = [28, 29]                      # m > 0
# fp16 W chunk sizes (k-tiles, sum = 57)
W_CH = [2, 2, 4, 4, 8, 8, 8, 8, 8, 5]
# fp8 w8 chunk sizes (k-pairs, sum = 28)
W8_CH = [2, 2, 4, 4, 8, 8]
# per-m xl8 chunk sizes (k-pairs, sum = 28)
XL_CH0 = [2, 6, 10, 10]
XL_CH = [14, 14]


def _offs(ch):
    return [sum(ch[:i]) for i in range(len(ch))]


def _build():
    nc = bacc.Bacc(trn_type="TRN2")
    XH = nc.dram_tensor("xh", [NM, 128, KP, MT], f16, kind="ExternalInput")
    XL = nc.dram_tensor("xl", [NM, 128, K2, 2 * MT], f8, kind="ExternalInput")
    WD = nc.dram_tensor("wd", [128, KP, 2 * E], f16, kind="ExternalInput")
    W8 = nc.dram_tensor("w8", [128, K2, 2 * E], f8, kind="ExternalInput")
    C = nc.dram_tensor("c", [128, 2 * E], f32, kind="ExternalInput")
    OIDX = nc.dram_tensor("oidx", [TLOC, TOP_K], i32, kind="ExternalOutput")
    OW = nc.dram_tensor("ow", [TLOC, TOP_K], f32, kind="ExternalOutput")

    with TileContext(nc) as tc:
        with (
            tc.tile_pool(name="const", bufs=1) as cpool,
            tc.tile_pool(name="xh", bufs=2) as xhpool,
            tc.tile_pool(name="xl", bufs=2) as xlpool,
            tc.tile_pool(name="s2", bufs=2) as s2,
            tc.tile_pool(name="pw", bufs=2, space="PSUM") as ppw,
            tc.tile_pool(name="pd", bufs=2, space="PSUM") as ppd,
            tc.tile_pool(name="pw1", bufs=1, space="PSUM") as ppw1,
            tc.tile_pool(name="pd1", bufs=1, space="PSUM") as ppd1,
        ):
            # ---- resident constants / weights (scalar-engine DMA queue) ----
            c_sb = cpool.tile([128, 2 * E], f32, name="c_sb")
            nc.scalar.dma_start(c_sb[:, :], C[:, :])
            b_rep = c_sb[:, 0:E]
            eb_rep = c_sb[:, E:2 * E]

            w_off = _offs(W_CH)
            w8_off = _offs(W8_CH)
            w_sbs = [None] * len(W_CH)
            w8_sbs = [None] * len(W8_CH)
            # issue in k-consumption order, interleaving fp16 and fp8 chunks
            wq = [("w", i) for i in range(len(W_CH))]
            w8q = [("w8", i) for i in range(len(W8_CH))]
            merged = []
            wi = w8i = 0
            for k in range(KP):
                while wi < len(W_CH) and w_off[wi] <= k:
                    merged.append(wq[wi]); wi += 1
                while w8i < len(W8_CH) and 2 * w8_off[w8i] + 1 <= k:
                    merged.append(w8q[w8i]); w8i += 1
            for kind, i in merged:
                if kind == "w":
                    nk = W_CH[i]
                    t = cpool.tile([128, nk * 2 * E], f16, name=f"w_sb{i}")
                    nc.scalar.dma_start(
                        t.rearrange("p (k e) -> p k e", k=nk),
                        WD[:, w_off[i]:w_off[i] + nk, :])
                    w_sbs[i] = t
                else:
                    nj = W8_CH[i]
                    t = cpool.tile([128, nj * 2 * E], f8, name=f"w8_sb{i}")
                    nc.scalar.dma_start(
                        t.rearrange("p (j e) -> p j e", j=nj),
                        W8[:, w8_off[i]:w8_off[i] + nj, :])
                    w8_sbs[i] = t

            wmap = []
            for ci, n in enumerate(W_CH):
                wmap += [(ci, j) for j in range(n)]
            w8map = []
            for ci, n in enumerate(W8_CH):
                w8map += [(ci, j) for j in range(n)]

            # ---- x loads ----
            def load_xh(m, i, offs, ch):
                t = xhpool.tile([128, ch[i] * MT], f16, tag=f"xh{i}",
                                name=f"xh{i}_{m}")
                nc.sync.dma_start(
                    t.rearrange("p (k t) -> p k t", k=ch[i]),
                    XH[m, :, offs[i]:offs[i] + ch[i], :])
                return t

            def load_xl(m, i, offs, ch):
                t = xlpool.tile([128, ch[i] * 2 * MT], f8, tag=f"xl{i}",
                                name=f"xl{i}_{m}")
                nc.gpsimd.dma_start(
                    t.rearrange("p (j t) -> p j t", j=ch[i]),
                    XL[m, :, offs[i]:offs[i] + ch[i], :])
                return t

            xh0_off, xh_off = _offs(XH_CH0), _offs(XH_CH)
            xl0_off, xl_off = _offs(XL_CH0), _offs(XL_CH)

            def xh_maps(ch):
                mp = []
                for ci, n in enumerate(ch):
                    mp += [(ci, j) for j in range(n)]
                return mp

            # preload m0 + m1
            x_pre = {}
            for m, (cho, cho_off) in ((0, (XH_CH0, xh0_off)), (1, (XH_CH, xh_off))):
                x_pre[m] = ([load_xh(m, i, cho_off, cho) for i in range(len(cho))],
                            xh_maps(cho))
            xl_pre = {}
            for m, (cho, cho_off) in ((0, (XL_CH0, xl0_off)), (1, (XL_CH, xl_off))):
                xl_pre[m] = ([load_xl(m, i, cho_off, cho) for i in range(len(cho))],
                             xh_maps(cho))

            def stage2(m, pw, pd):
                t0 = s2.tile([128, E], f32, tag="t0", name=f"t0_{m}")
                nc.scalar.mul(t0[:, :], pw[:, E:2 * E], 1.0 / SCW)
                l1 = s2.tile([128, E], f32, tag="l1", name=f"l1_{m}")
                nc.vector.scalar_tensor_tensor(
                    l1[:, :], pd[:, :], DSC, t0[:, :],
                    op0=AOT.mult, op1=AOT.add)
                lg = s2.tile([128, E], f32, tag="lg", name=f"lg_{m}")
                nc.vector.scalar_tensor_tensor(
                    lg[:, :], pw[:, 0:E], 1.0, l1[:, :],
                    op0=AOT.mult, op1=AOT.add)
                s = s2.tile([128, E], f32, tag="s", name=f"s_{m}")
                nc.scalar.activation(s[:, :], lg[:, :], ACTF.Sigmoid)
                sfc = s2.tile([128, E], f32, tag="sfc", name=f"sfc_{m}")
                nc.vector.tensor_add(sfc[:, :], s[:, :], eb_rep)

                gmax = s2.tile([128, 8 * N_GROUP], f32, tag="gmax", name=f"gmax_{m}")
                for g in range(N_GROUP):
                    nc.vector.max(out=gmax[:, g * 8:(g + 1) * 8],
                                  in_=sfc[:, g * EG:(g + 1) * EG])
                gm3 = gmax.rearrange("p (g c) -> p g c", c=8)
                gs = s2.tile([128, N_GROUP], f32, tag="gs", name=f"gs_{m}")
                nc.vector.tensor_add(gs.unsqueeze(2), gm3[:, :, 0:1], gm3[:, :, 1:2])
                g8 = s2.tile([128, 8], f32, tag="g8", name=f"g8_{m}")
                nc.vector.max(out=g8[:, :], in_=gs[:, :])
                gmask = s2.tile([128, N_GROUP], f32, tag="gmask", name=f"gmask_{m}")
                nc.vector.tensor_scalar(gmask[:, :], gs[:, :],
                                        g8[:, TOPK_GROUP - 1:TOPK_GROUP], None,
                                        op0=AOT.is_ge)
                gmm = s2.tile([128, 8 * N_GROUP], f32, tag="gmm", name=f"gmm_{m}")
                nc.vector.tensor_mul(
                    gmm.rearrange("p (g c) -> p g c", c=8),
                    gm3,
                    gmask.unsqueeze(2).to_broadcast([128, N_GROUP, 8]))
                m8 = s2.tile([128, 8], f32, tag="m8", name=f"m8_{m}")
                nc.vector.max(out=m8[:, :], in_=gmm[:, :])

                masked = s2.tile([128, E], f32, tag="masked", name=f"masked_{m}")
                nc.vector.tensor_mul(
                    masked.rearrange("p (g c) -> p g c", c=EG),
                    sfc.rearrange("p (g c) -> p g c", c=EG),
                    gmask.unsqueeze(2).to_broadcast([128, N_GROUP, EG]))
                i8 = s2.tile([128, 8], u32, tag="i8", name=f"i8_{m}")
                nc.vector.max_index(out=i8[:, :], in_max=m8[:, :],
                                    in_values=masked[:, :])
                nc.sync.dma_start(OIDX[m * MT:(m + 1) * MT, :], i8.bitcast(i32))

                # eb[idx_j] via match-accumulate; w8 = m8 - eb[idx]
                eb8 = s2.tile([128, 8], f32, tag="eb8", name=f"eb8_{m}")
                scr = s2.tile([128, E], f32, tag="scr", name=f"scr_{m}")
                for j in range(TOP_K):
                    nc.vector.scalar_tensor_tensor(
                        scr[:, :], sfc[:, :], m8[:, j:j + 1], eb_rep,
                        op0=AOT.is_equal, op1=AOT.mult,
                        accum_out=eb8[:, j:j + 1])
                w8v = s2.tile([128, 8], f32, tag="w8v", name=f"w8v_{m}")
                nc.vector.tensor_tensor(w8v[:, :], m8[:, :], eb8[:, :],
                                        op=AOT.subtract)
                rs = s2.tile([128, 1], f32, tag="rs", name=f"rs_{m}")
                nc.vector.tensor_reduce(rs[:, :], w8v[:, :],
                                        axis=mybir.AxisListType.X, op=AOT.add)
                rc = s2.tile([128, 1], f32, tag="rc", name=f"rc_{m}")
                nc.vector.reciprocal(rc[:, :], rs[:, :])
                wo = s2.tile([128, 8], f32, tag="wo", name=f"wo_{m}")
                nc.vector.tensor_scalar(wo[:, :], w8v[:, :], rc[:, 0:1], SCALE,
                                        op0=AOT.mult, op1=AOT.mult)
                nc.sync.dma_start(OW[m * MT:(m + 1) * MT, :], wo[:, :])

            groups = [[0, 1]] + [[m] for m in range(2, NM)]
            for group in groups:
                pss = {}
                for gi, m in enumerate(group):
                    pw = (ppw if gi == 0 else ppw1).tile(
                        [128, 2 * E], f32, tag=f"pw{gi}", name=f"pw_{m}")
                    pd = (ppd if gi == 0 else ppd1).tile(
                        [128, E], f32, tag=f"pd{gi}", name=f"pd_{m}")
                    pss[m] = (pw, pd)
                xts = {}
                for m in group:
                    if m in x_pre:
                        xts[m] = (x_pre[m], xl_pre[m])
                    else:
                        xts[m] = (
                            ([load_xh(m, i, xh_off, XH_CH)
                              for i in range(len(XH_CH))], xh_maps(XH_CH)),
                            ([load_xl(m, i, xl_off, XL_CH)
                              for i in range(len(XL_CH))], xh_maps(XL_CH)),
                        )
                for k in range(KP):
                    wc, kw = wmap[k]
                    wt = w_sbs[wc]
                    w_ap = wt[:, kw * 2 * E:(kw + 1) * 2 * E]
                    for m in group:
                        (xhl, xhm), _ = xts[m]
                        xc, kl = xhm[k]
                        xh_ap = xhl[xc][:, kl * MT:(kl + 1) * MT]
                        pw, _ = pss[m]
                        nc.tensor.matmul(pw[:, :], xh_ap, w_ap,
                                         start=(k == 0), stop=(k == KP - 1))
                    if k % 2 == 1 and k < KT:
                        j = k // 2
                        w8c, jw = w8map[j]
                        w8t = w8_sbs[w8c]
                        w8_ap = w8t[:, jw * 2 * E:(jw + 1) * 2 * E].rearrange(
                            "p (a e) -> p a e", a=2)
                        for m in group:
                            _, (xll, xlm) = xts[m]
                            xc, jl = xlm[j]
                            xl_ap = xll[xc][:, jl * 2 * MT:(jl + 1) * 2 * MT].rearrange(
                                "p (a t) -> p a t", a=2)
                            _, pd = pss[m]
                            nc.tensor.matmul(pd[:, :], xl_ap, w8_ap,
                                             start=(j == 0), stop=(j == K2 - 1),
                                             perf_mode=DR)
                for m in group:
                    stage2(m, *pss[m])

    nc.finalize()
    return nc


def _pack_x(x_shard: np.ndarray):
    """[TLOC, H] f32 -> (xh [NM,128,KP,MT] f16, xl8 [NM,128,K2,2*MT] f8e4)."""
    xT = np.ascontiguousarray(x_shard.T)               # [H, TLOC]
    xh = xT.astype(np.float16)
    xl = (xT - xh.astype(np.float32)) * SX8
    xh_t = np.zeros((NM, 128, KP, MT), np.float16)
    xh_t[:, :, :KT, :] = xh.reshape(KT, 128, NM, MT).transpose(2, 1, 0, 3)
    xh_t[:, 0, KT, :] = 1.0                            # bias row
    xl8 = xl.astype(NPF8).reshape(K2, 2, 128, NM, MT).transpose(3, 2, 0, 1, 4)
    xl8 = np.ascontiguousarray(xl8.reshape(NM, 128, K2, 2 * MT))
    return np.ascontiguousarray(xh_t), xl8


def _pack_w(W: np.ndarray, b: np.ndarray):
    """[E,H] f32 -> (wd [128,KP,2E] f16 = [wh|wl*2^11] + bias row,
                     w8 [128,K2,2E] f8e4 = w*2^11 in k-pairs)."""
    wT = np.ascontiguousarray(W.T)                     # [H, E]
    wh = wT.astype(np.float16)
    wl = ((wT - wh.astype(np.float32)) * SCW).astype(np.float16)
    wd = np.zeros((128, KP, 2 * E), np.float16)
    wd[:, :KT, :E] = wh.reshape(KT, 128, E).transpose(1, 0, 2)
    wd[:, :KT, E:] = wl.reshape(KT, 128, E).transpose(1, 0, 2)
    b16 = b.astype(np.float16)
    wd[0, KT, :E] = b16
    wd[0, KT, E:] = ((b - b16.astype(np.float32)) * SCW).astype(np.float16)
    w8 = (wT * SW8).astype(NPF8).reshape(K2, 2, 128, E).transpose(2, 0, 1, 3)
    w8 = np.ascontiguousarray(w8.reshape(128, K2, 2 * E))
    return np.ascontiguousarray(wd), w8


def prepare_in_maps(hidden_states, W, b, e_score_correction_bias):
    wd, w8 = _pack_w(np.asarray(W, np.float32), np.asarray(b, np.float32))
    consts = np.empty((128, 2 * E), np.float32)
    consts[:, 0:E] = np.asarray(b, np.float32)[None, :]
    consts[:, E:2 * E] = np.asarray(e_score_correction_bias, np.float32)[None, :]
    hs = np.asarray(hidden_states, np.float32)
    in_maps = []
    for c in range(NCORES):
        xh_t, xl8 = _pack_x(hs[c * TLOC:(c + 1) * TLOC])
        in_maps.append({"xh": xh_t, "xl": xl8, "wd": wd, "w8": w8, "c": consts})
    return in_maps


def get_prog():
    global _PROG
    if _PROG is None:
        _PROG = _build()
    return _PROG


def kernel(hidden_states, W, b, e_score_correction_bias):
    nc = get_prog()
    in_maps = prepare_in_maps(hidden_states, W, b, e_score_correction_bias)
    res = bass_utils.run_bass_kernel_spmd(nc, in_maps, core_ids=list(range(NCORES)))
    idx = np.concatenate([res.results[c]["oidx"] for c in range(NCORES)], axis=0)
    wts = np.concatenate([res.results[c]["ow"] for c in range(NCORES)], axis=0)
    return idx.astype(np.int32), wts.astype(np.float32)


# revision 9
# speedup vs baseline: 1.1179x; 1.1179x over previous
"""DeepseekV3 top-k router kernel for 8x Trainium2 NeuronCores.

Strategy (v2):
  - Token dim (8192) sharded 8 ways; router weight replicated per core.
  - logits = hidden @ W.T decomposed as xh*wh + xh*wl + xl*w:
      * one 512-wide fp16 matmul per k-tile streams [wh | wl*2^11] through
        stationary xh -> psum[0:256] (hi) and psum[256:512] (cross1*2^11).
      * one fp8e4 DoubleRow matmul per k-PAIR contracts xl8 (=xl*2^16) against
        w8 (=w*2^11) for two k-tiles at once -> psum_dr (cross2*2^27).
    Bias b is folded in exactly via a 57th k-tile (ones row x [b16|b_res*2^11]).
    Empirically (fixed-seed inputs) this matches the fp32 reference's top-k
    indices as well as a full 3-pass fp16 scheme (1 near-tie row in 8192).
  - sigmoid + grouped top-k + weight extraction run on ACT/DVE per 128-token
    tile, overlapped with the next tile's matmuls. Weight gather uses
    scalar_tensor_tensor accumulate: eb[idx_j] = sum((sfc==m8_j)*eb).
  - DMA: x ships as fp16 hi (2B) + fp8 lo (1B); descriptors spread across
    sync/scalar/gpsimd engine queues so startup is not serialized on one
    sequencer.
"""

import numpy as np
import ml_dtypes

import concourse.bacc as bacc
import concourse.mybir as mybir
from concourse.tile import TileContext
from concourse import bass_utils

H = 7168
E = 256
T = 8192
NCORES = 8
TLOC = T // NCORES          # 1024 tokens per core
MT = 128                    # tokens per tile (PSUM partition dim)
NM = TLOC // MT             # 8 token tiles per core
KT = H // 128               # 56 contraction tiles
KP = KT + 1                 # +1 bias tile for the fp16 pass
K2 = KT // 2                # 28 DoubleRow k-pairs
TOP_K = 8
N_GROUP = 8
TOPK_GROUP = 4
EG = E // N_GROUP           # 32 experts per group
SCALE = 2.5
SCW = 2048.0                # 2^11 wl pre-scale (fp16 cross half)
SX8 = 2.0 ** 16             # xl -> fp8 pre-scale
SW8 = 2.0 ** 11             # w -> fp8 pre-scale
DSC = 1.0 / (SX8 * SW8)     # descale for the DoubleRow psum

f32 = mybir.dt.float32
f16 = mybir.dt.float16
f8 = mybir.dt.float8e4
u32 = mybir.dt.uint32
i32 = mybir.dt.int32
AOT = mybir.AluOpType
ACTF = mybir.ActivationFunctionType
DR = mybir.MatmulPerfMode.DoubleRow
NPF8 = ml_dtypes.float8_e4m3

_PROG = None

# per-m xh chunk sizes in k-tiles (sum = KP = 57)
XH_CH0 = [2, 6, 8, 8, 8, 8, 8, 9]     # m == 0 (fine-grained startup)
XH_CH = [28, 29]                      # m > 0
# fp16 W chunk sizes (k-tiles, sum = 57)
W_CH = [1, 1, 2, 2, 4, 4, 8, 8, 8, 8, 8, 3]
# fp8 w8 chunk sizes (k-pairs, sum = 28)
W8_CH = [1, 1, 2, 4, 4, 8, 8]
# per-m xl8 chunk sizes (k-pairs, sum = 28)
XL_CH0 = [2, 6, 10, 10]
XL_CH = [14, 14]


def _offs(ch):
    return [sum(ch[:i]) for i in range(len(ch))]


def _build():
    nc = bacc.Bacc(trn_type="TRN2")
    XH = nc.dram_tensor("xh", [NM, 128, KP, MT], f16, kind="ExternalInput")
    XL = nc.dram_tensor("xl", [NM, 128, K2, 2 * MT], f8, kind="ExternalInput")
    WD = nc.dram_tensor("wd", [128, KP, 2 * E], f16, kind="ExternalInput")
    W8 = nc.dram_tensor("w8", [128, K2, 2 * E], f8, kind="ExternalInput")
    C = nc.dram_tensor("c", [128, 2 * E], f32, kind="ExternalInput")
    OIDX = nc.dram_tensor("oidx", [TLOC, TOP_K], i32, kind="ExternalOutput")
    OW = nc.dram_tensor("ow", [TLOC, TOP_K], f32, kind="ExternalOutput")

    with TileContext(nc) as tc:
        with (
            tc.tile_pool(name="const", bufs=1) as cpool,
            tc.tile_pool(name="xh0", bufs=1) as xh0pool,
            tc.tile_pool(name="xl0", bufs=1) as xl0pool,
            tc.tile_pool(name="xh", bufs=2) as xhpool,
            tc.tile_pool(name="xl", bufs=2) as xlpool,
            tc.tile_pool(name="s2", bufs=2) as s2,
            tc.tile_pool(name="pw", bufs=2, space="PSUM") as ppw,
            tc.tile_pool(name="pd", bufs=2, space="PSUM") as ppd,
            tc.tile_pool(name="pw1", bufs=1, space="PSUM") as ppw1,
            tc.tile_pool(name="pd1", bufs=1, space="PSUM") as ppd1,
        ):
            # ---- resident constants / weights (scalar-engine DMA queue) ----
            c_sb = cpool.tile([128, 2 * E], f32, name="c_sb")
            nc.scalar.dma_start(c_sb[:, :], C[:, :])
            b_rep = c_sb[:, 0:E]
            eb_rep = c_sb[:, E:2 * E]

            w_off = _offs(W_CH)
            w8_off = _offs(W8_CH)
            w_sbs = [None] * len(W_CH)
            w8_sbs = [None] * len(W8_CH)
            # issue in k-consumption order, interleaving fp16 and fp8 chunks
            wq = [("w", i) for i in range(len(W_CH))]
            w8q = [("w8", i) for i in range(len(W8_CH))]
            merged = []
            wi = w8i = 0
            for k in range(KP):
                while wi < len(W_CH) and w_off[wi] <= k:
                    merged.append(wq[wi]); wi += 1
                while w8i < len(W8_CH) and 2 * w8_off[w8i] + 1 <= k:
                    merged.append(w8q[w8i]); w8i += 1
            for kind, i in merged:
                if kind == "w":
                    nk = W_CH[i]
                    t = cpool.tile([128, nk * 2 * E], f16, name=f"w_sb{i}")
                    nc.scalar.dma_start(
                        t.rearrange("p (k e) -> p k e", k=nk),
                        WD[:, w_off[i]:w_off[i] + nk, :])
                    w_sbs[i] = t
                else:
                    nj = W8_CH[i]
                    t = cpool.tile([128, nj * 2 * E], f8, name=f"w8_sb{i}")
                    nc.scalar.dma_start(
                        t.rearrange("p (j e) -> p j e", j=nj),
                        W8[:, w8_off[i]:w8_off[i] + nj, :])
                    w8_sbs[i] = t

            wmap = []
            for ci, n in enumerate(W_CH):
                wmap += [(ci, j) for j in range(n)]
            w8map = []
            for ci, n in enumerate(W8_CH):
                w8map += [(ci, j) for j in range(n)]

            # ---- x loads ----
            def load_xh(m, i, offs, ch):
                pool = xh0pool if m == 0 else xhpool
                t = pool.tile([128, ch[i] * MT], f16, tag=f"xh{m==0}{i}",
                              name=f"xh{i}_{m}")
                nc.sync.dma_start(
                    t.rearrange("p (k t) -> p k t", k=ch[i]),
                    XH[m, :, offs[i]:offs[i] + ch[i], :])
                return t

            def load_xl(m, i, offs, ch):
                pool = xl0pool if m == 0 else xlpool
                t = pool.tile([128, ch[i] * 2 * MT], f8, tag=f"xl{m==0}{i}",
                              name=f"xl{i}_{m}")
                nc.gpsimd.dma_start(
                    t.rearrange("p (j t) -> p j t", j=ch[i]),
                    XL[m, :, offs[i]:offs[i] + ch[i], :])
                return t

            xh0_off, xh_off = _offs(XH_CH0), _offs(XH_CH)
            xl0_off, xl_off = _offs(XL_CH0), _offs(XL_CH)

            def xh_maps(ch):
                mp = []
                for ci, n in enumerate(ch):
                    mp += [(ci, j) for j in range(n)]
                return mp

            # preload m0 + m1
            x_pre = {}
            for m, (cho, cho_off) in ((0, (XH_CH0, xh0_off)), (1, (XH_CH, xh_off))):
                x_pre[m] = ([load_xh(m, i, cho_off, cho) for i in range(len(cho))],
                            xh_maps(cho))
            xl_pre = {}
            for m, (cho, cho_off) in ((0, (XL_CH0, xl0_off)), (1, (XL_CH, xl_off))):
                xl_pre[m] = ([load_xl(m, i, cho_off, cho) for i in range(len(cho))],
                             xh_maps(cho))

            def stage2(m, pw, pd):
                t0 = s2.tile([128, E], f32, tag="t0", name=f"t0_{m}")
                nc.scalar.mul(t0[:, :], pw[:, E:2 * E], 1.0 / SCW)
                l1 = s2.tile([128, E], f32, tag="l1", name=f"l1_{m}")
                nc.vector.scalar_tensor_tensor(
                    l1[:, :], pd[:, :], DSC, t0[:, :],
                    op0=AOT.mult, op1=AOT.add)
                lg = s2.tile([128, E], f32, tag="lg", name=f"lg_{m}")
                nc.vector.scalar_tensor_tensor(
                    lg[:, :], pw[:, 0:E], 1.0, l1[:, :],
                    op0=AOT.mult, op1=AOT.add)
                s = s2.tile([128, E], f32, tag="s", name=f"s_{m}")
                nc.scalar.activation(s[:, :], lg[:, :], ACTF.Sigmoid)
                sfc = s2.tile([128, E], f32, tag="sfc", name=f"sfc_{m}")
                nc.vector.tensor_add(sfc[:, :], s[:, :], eb_rep)

                gmax = s2.tile([128, 8 * N_GROUP], f32, tag="gmax", name=f"gmax_{m}")
                for g in range(N_GROUP):
                    nc.vector.max(out=gmax[:, g * 8:(g + 1) * 8],
                                  in_=sfc[:, g * EG:(g + 1) * EG])
                gm3 = gmax.rearrange("p (g c) -> p g c", c=8)
                gs = s2.tile([128, N_GROUP], f32, tag="gs", name=f"gs_{m}")
                nc.vector.tensor_add(gs.unsqueeze(2), gm3[:, :, 0:1], gm3[:, :, 1:2])
                g8 = s2.tile([128, 8], f32, tag="g8", name=f"g8_{m}")
                nc.vector.max(out=g8[:, :], in_=gs[:, :])
                gmask = s2.tile([128, N_GROUP], f32, tag="gmask", name=f"gmask_{m}")
                nc.vector.tensor_scalar(gmask[:, :], gs[:, :],
                                        g8[:, TOPK_GROUP - 1:TOPK_GROUP], None,
                                        op0=AOT.is_ge)
                gmm = s2.tile([128, 8 * N_GROUP], f32, tag="gmm", name=f"gmm_{m}")
                nc.vector.tensor_mul(
                    gmm.rearrange("p (g c) -> p g c", c=8),
                    gm3,
                    gmask.unsqueeze(2).to_broadcast([128, N_GROUP, 8]))
                m8 = s2.tile([128, 8], f32, tag="m8", name=f"m8_{m}")
                nc.vector.max(out=m8[:, :], in_=gmm[:, :])

                masked = s2.tile([128, E], f32, tag="masked", name=f"masked_{m}")
                nc.vector.tensor_mul(
                    masked.rearrange("p (g c) -> p g c", c=EG),
                    sfc.rearrange("p (g c) -> p g c", c=EG),
                    gmask.unsqueeze(2).to_broadcast([128, N_GROUP, EG]))
                i8 = s2.tile([128, 8], u32, tag="i8", name=f"i8_{m}")
                nc.vector.max_index(out=i8[:, :], in_max=m8[:, :],
                                    in_values=masked[:, :])
                nc.gpsimd.dma_start(OIDX[m * MT:(m + 1) * MT, :], i8.bitcast(i32))

                # eb[idx_j] via one-hot match rows + one reduction
                scr = s2.tile([128, TOP_K * E], f32, tag="scr", name=f"scr_{m}")
                for j in range(TOP_K):
                    nc.vector.scalar_tensor_tensor(
                        scr[:, j * E:(j + 1) * E], sfc[:, :], m8[:, j:j + 1],
                        eb_rep, op0=AOT.is_equal, op1=AOT.mult)
                eb8 = s2.tile([128, 8], f32, tag="eb8", name=f"eb8_{m}")
                nc.vector.tensor_reduce(eb8[:, :],
                                        scr.rearrange("p (j e) -> p j e", j=TOP_K),
                                        axis=mybir.AxisListType.X, op=AOT.add)
                w8v = s2.tile([128, 8], f32, tag="w8v", name=f"w8v_{m}")
                nc.vector.tensor_tensor(w8v[:, :], m8[:, :], eb8[:, :],
                                        op=AOT.subtract)
                rs = s2.tile([128, 1], f32, tag="rs", name=f"rs_{m}")
                nc.vector.tensor_reduce(rs[:, :], w8v[:, :],
                                        axis=mybir.AxisListType.X, op=AOT.add)
                rc = s2.tile([128, 1], f32, tag="rc", name=f"rc_{m}")
                nc.vector.reciprocal(rc[:, :], rs[:, :])
                wo = s2.tile([128, 8], f32, tag="wo", name=f"wo_{m}")
                nc.vector.tensor_scalar(wo[:, :], w8v[:, :], rc[:, 0:1], SCALE,
                                        op0=AOT.mult, op1=AOT.mult)
                nc.gpsimd.dma_start(OW[m * MT:(m + 1) * MT, :], wo[:, :])

            groups = [[0, 1]] + [[m] for m in range(2, NM)]
            for group in groups:
                pss = {}
                for gi, m in enumerate(group):
                    pw = (ppw if gi == 0 else ppw1).tile(
                        [128, 2 * E], f32, tag=f"pw{gi}", name=f"pw_{m}")
                    pd = (ppd if gi == 0 else ppd1).tile(
                        [128, E], f32, tag=f"pd{gi}", name=f"pd_{m}")
                    pss[m] = (pw, pd)
                xts = {}
                for m in group:
                    if m in x_pre:
                        xts[m] = (x_pre[m], xl_pre[m])
                    else:
                        xts[m] = (
                            ([load_xh(m, i, xh_off, XH_CH)
                              for i in range(len(XH_CH))], xh_maps(XH_CH)),
                            ([load_xl(m, i, xl_off, XL_CH)
                              for i in range(len(XL_CH))], xh_maps(XL_CH)),
                        )
                for k in range(KP):
                    wc, kw = wmap[k]
                    wt = w_sbs[wc]
                    w_ap = wt[:, kw * 2 * E:(kw + 1) * 2 * E]
                    for m in group:
                        (xhl, xhm), _ = xts[m]
                        xc, kl = xhm[k]
                        xh_ap = xhl[xc][:, kl * MT:(kl + 1) * MT]
                        pw, _ = pss[m]
                        nc.tensor.matmul(pw[:, :], xh_ap, w_ap,
                                         start=(k == 0), stop=(k == KP - 1))
                    if k % 2 == 1 and k < KT:
                        j = k // 2
                        w8c, jw = w8map[j]
                        w8t = w8_sbs[w8c]
                        w8_ap = w8t[:, jw * 2 * E:(jw + 1) * 2 * E].rearrange(
                            "p (a e) -> p a e", a=2)
                        for m in group:
                            _, (xll, xlm) = xts[m]
                            xc, jl = xlm[j]
                            xl_ap = xll[xc][:, jl * 2 * MT:(jl + 1) * 2 * MT].rearrange(
                                "p (a t) -> p a t", a=2)
                            _, pd = pss[m]
                            nc.tensor.matmul(pd[:, :], xl_ap, w8_ap,
                                             start=(j == 0), stop=(j == K2 - 1),
                                             perf_mode=DR)
                for m in group:
                    stage2(m, *pss[m])

    nc.finalize()
    return nc


def _pack_x(x_shard: np.ndarray):
    """[TLOC, H] f32 -> (xh [NM,128,KP,MT] f16, xl8 [NM,128,K2,2*MT] f8e4)."""
    xT = np.ascontiguousarray(x_shard.T)               # [H, TLOC]
    xh = xT.astype(np.float16)
    xl = (xT - xh.astype(np.float32)) * SX8
    xh_t = np.zeros((NM, 128, KP, MT), np.float16)
    xh_t[:, :, :KT, :] = xh.reshape(KT, 128, NM, MT).transpose(2, 1, 0, 3)
    xh_t[:, 0, KT, :] = 1.0                            # bias row
    xl8 = xl.astype(NPF8).reshape(K2, 2, 128, NM, MT).transpose(3, 2, 0, 1, 4)
    xl8 = np.ascontiguousarray(xl8.reshape(NM, 128, K2, 2 * MT))
    return np.ascontiguousarray(xh_t), xl8


def _pack_w(W: np.ndarray, b: np.ndarray):
    """[E,H] f32 -> (wd [128,KP,2E] f16 = [wh|wl*2^11] + bias row,
                     w8 [128,K2,2E] f8e4 = w*2^11 in k-pairs)."""
    wT = np.ascontiguousarray(W.T)                     # [H, E]
    wh = wT.astype(np.float16)
    wl = ((wT - wh.astype(np.float32)) * SCW).astype(np.float16)
    wd = np.zeros((128, KP, 2 * E), np.float16)
    wd[:, :KT, :E] = wh.reshape(KT, 128, E).transpose(1, 0, 2)
    wd[:, :KT, E:] = wl.reshape(KT, 128, E).transpose(1, 0, 2)
    b16 = b.astype(np.float16)
    wd[0, KT, :E] = b16
    wd[0, KT, E:] = ((b - b16.astype(np.float32)) * SCW).astype(np.float16)
    w8 = (wT * SW8).astype(NPF8).reshape(K2, 2, 128, E).transpose(2, 0, 1, 3)
    w8 = np.ascontiguousarray(w8.reshape(128, K2, 2 * E))
    return np.ascontiguousarray(wd), w8


def prepare_in_maps(hidden_states, W, b, e_score_correction_bias):
    wd, w8 = _pack_w(np.asarray(W, np.float32), np.asarray(b, np.float32))
    consts = np.empty((128, 2 * E), np.float32)
    consts[:, 0:E] = np.asarray(b, np.float32)[None, :]
    consts[:, E:2 * E] = np.asarray(e_score_correction_bias, np.float32)[None, :]
    hs = np.asarray(hidden_states, np.float32)
    in_maps = []
    for c in range(NCORES):
        xh_t, xl8 = _pack_x(hs[c * TLOC:(c + 1) * TLOC])
        in_maps.append({"xh": xh_t, "xl": xl8, "wd": wd, "w8": w8, "c": consts})
    return in_maps


def get_prog():
    global _PROG
    if _PROG is None:
        _PROG = _build()
    return _PROG


def kernel(hidden_states, W, b, e_score_correction_bias):
    nc = get_prog()
    in_maps = prepare_in_maps(hidden_states, W, b, e_score_correction_bias)
    res = bass_utils.run_bass_kernel_spmd(nc, in_maps, core_ids=list(range(NCORES)))
    idx = np.concatenate([res.results[c]["oidx"] for c in range(NCORES)], axis=0)
    wts = np.concatenate([res.results[c]["ow"] for c in range(NCORES)], axis=0)
    return idx.astype(np.int32), wts.astype(np.float32)
